# revision 46
# baseline (speedup 1.0000x reference)
"""Trainium2 Bass kernel for nn_BinaryMLP (BitNet-ternary SwiGLU MLP).

reference math (fp32):
    s_i = mean(|w_i|)            (per-tensor scalar, i in {1,3,2})
    wq_i = clip(round(w_i/s_i), -1, 1) * s_i     (ternary * scale)
    h1 = x @ w1q.T ; h3 = x @ w3q.T
    y  = (silu(h1) * h3) @ w2q.T

Strategy (8 cores, data-parallel over the 16384 tokens):
  - host: pad H 5461->5504, transpose x / w1 / w3 / w2 into contraction-major
    layouts (pure layout work, no arithmetic), split tokens 8 ways, and give
    each core a distinct 1/8 row-slice of each weight tensor.
  - device (per core, identical SPMD program):
      phase A: w1/w3 shard resident in SBUF (read once); |w| partial sums
               (DVE abs-reduce) -> gpsimd partition-reduce -> tiny 8-core
               AllReduce -> ternarization thresholds +-s/2.  The first two
               h-chunks are allgathered RAW in bf16 (queued on the CC
               stream around the AllReduce, no threshold dependency) and
               ternarized locally on every core straight into SBUF weight
               tiles, so phase B starts as soon as the scales exist.
               Remaining chunks are ternarized at the source (fp8 {-2,0,2})
               and allgathered while phase B runs.  Meanwhile x is loaded
               and split exactly into two fp8 terms x = x_hi + x_lo
               (x_hi = fp8(x), x_lo = fp8(x - x_hi)) so phase B runs
               fp8xfp8 DoubleRow matmuls: same PE cycles as bf16, about
               half the PE power (the chip is power-brake limited).
      phase B: h1/h3 DoubleRow fp8 matmuls vs resident x_hi/x_lo (weight
               pairs stationary, 8 psum banks), fused epilogue
               g = silu((s1/2) z1) * (s3 s2/4 * z3) in one ACT + one DVE
               op -> bf16 -> DRAM.
      phase C: y[m,d] = sum_h g[h,m] t2[h,d], g stationary bf16, w2 fp8
               moving, fp32 PSUM, bf16 output (host upcasts losslessly
               concatenation-side).
  - host: concatenate the 8 token shards, reshape to [4, 4096, 2048].

All arithmetic (scales, ternarization, matmuls) happens on device; the host
only reshapes / transposes / pads / slices / concatenates / upcasts.
"""

import sys
from contextlib import ExitStack

import numpy as np

if "/opt/trn_rl_repo" not in sys.path:
    sys.path.insert(0, "/opt/trn_rl_repo")

import concourse.bass as bass  # noqa: E402,F401
import concourse.mybir as mybir  # noqa: E402
import concourse.tile as tile  # noqa: E402
from concourse import bacc  # noqa: E402
from concourse import bass_isa  # noqa: E402

F32 = mybir.dt.float32
BF16 = mybir.dt.bfloat16
FP8 = mybir.dt.float8e4
AF = mybir.ActivationFunctionType
ALU = mybir.AluOpType
AX = mybir.AxisListType
DR = mybir.MatmulPerfMode.DoubleRow

# Full problem geometry (hardcoded per contest rules).
B, S, D = 4, 4096, 2048
H_REAL = 5461
HP = 5504            # H padded to 43*128
N_CORES = 8
M = (B * S) // N_CORES   # tokens per core = 2048


def build_module(d=D, m=M, hp=HP, n_cores=N_CORES, h_real=H_REAL,
                 hb=4, w13_dt=FP8, w2q_dt=FP8, nwarm=16):
    """Build + compile the per-core SPMD Bass module."""
    kd = d // 128        # k-tiles over D
    ht = hp // 128       # h-tiles
    mc = m // 512        # m-chunks of 512 in phase B
    assert d % 128 == 0 and hp % 128 == 0 and m % 512 == 0
    n_true = h_real * d
    sw = d * hp // (n_cores * 128)   # w2-shard free elems per partition
    r13 = d // n_cores               # weight-slice rows (w1t/w3t)
    assert r13 % 128 == 0
    a13 = r13 // 128

    # h-tile chunks: chunks 0/1 go through the raw bf16 allgather (low
    # latency, threshold-free); 2+ are ternarized at source (fp8).
    sizes = [2, 3, 8, 8, 8, 8, 6]
    assert sum(sizes) == ht
    chunks = []
    t0 = 0
    for n in sizes:
        chunks.append((t0, n))
        t0 += n
    n_raw = 1
    c0w = sizes[0] * 128
    rawsplit = c0w           # wsh columns needed by the raw allgather

    nc = bacc.Bacc(
        "TRN2",
        target_bir_lowering=False,
        debug=False,
        num_devices=n_cores,
    )
    xT = nc.dram_tensor("xT", [d, m], F32, kind="ExternalInput").ap()
    wsh1 = nc.dram_tensor("wsh1", [r13, hp], F32, kind="ExternalInput").ap()
    wsh3 = nc.dram_tensor("wsh3", [r13, hp], F32, kind="ExternalInput").ap()
    wsh2 = nc.dram_tensor("wsh2", [128, sw], F32, kind="ExternalInput").ap()
    y = nc.dram_tensor("y", [m, d], BF16, kind="ExternalOutput").ap()

    xview = xT.rearrange("(k p) m -> p k m", p=128)
    v1 = wsh1.rearrange("(a p) h -> p a h", p=128)   # [128, a13, hp]
    v3 = wsh3.rearrange("(a p) h -> p a h", p=128)

    with tile.TileContext(nc) as tc:
        with ExitStack() as ctx:
            dram = ctx.enter_context(tc.tile_pool(name="dram", bufs=1, space="DRAM"))
            g_dram = dram.tile([hp, m], BF16, tag="g", name="g")
            cc_in = dram.tile([1, 8], F32, tag="cc_in", name="cc_in")
            cc_out = dram.tile([1, 8], F32, tag="cc_out", name="cc_out")
            g_rd = g_dram.rearrange("(k p) m -> p k m", p=128)

            agraw_in = []
            agraw_out = []
            for q in range(n_raw):
                cw = sizes[q] * 128
                agraw_in.append(dram.tile([r13, 2, cw], BF16,
                                          tag=f"agri{q}", name=f"agri{q}"))
                agraw_out.append(dram.tile([d, 2, cw], BF16,
                                           tag=f"agro{q}", name=f"agro{q}"))
            agin13 = [None] * n_raw
            agout13 = [None] * n_raw
            for q, (ct0, cnt) in enumerate(chunks):
                if q < n_raw:
                    continue
                agin13.append(dram.tile(
                    [r13, 2, cnt * 128], w13_dt,
                    tag=f"agi{q}", name=f"agi{q}"))
                agout13.append(dram.tile(
                    [d, 2, cnt * 128], w13_dt,
                    tag=f"ago{q}", name=f"ago{q}"))
            agin2 = dram.tile([128, sw], w2q_dt, tag="agi2", name="agi2")
            agout2 = dram.tile([hp, d], w2q_dt, tag="ago2", name="ago2")
            agout2_rd = agout2.rearrange("(k p) dd -> p k dd", p=128)

            pc = ctx.enter_context(tc.tile_pool(name="pconst", bufs=1))
            bias = {}
            # x_hi / x_lo fp8, resident through phase B (dead but allocated
            # in phase C — pool stack is LIFO)
            xp = ctx.enter_context(tc.tile_pool(name="xp", bufs=1))
            x_sb = xp.tile([128, 2 * kd, m], FP8, tag="x_sb", name="x_sb")
            # chunk-0 ternary tiles (locally quantized during phase A)
            c0p = ctx.enter_context(tc.tile_pool(name="c0p", bufs=1))
            sx = ctx.enter_context(ExitStack())
            zps = sx.enter_context(tc.tile_pool(name="zps", bufs=8,
                                                space="PSUM"))

            # ---------------- warmup: keep the PE spinning from t=0 ---------
            warm_w = pc.tile([128, 2, 128], FP8, tag="warm_w", name="warm_w")
            warm_x = pc.tile([128, 2, 512], FP8, tag="warm_x", name="warm_x")
            nc.vector.memset(warm_w, 0.125)
            nc.vector.memset(warm_x, 0.125)
            wz = zps.tile([128, 512], F32, tag="z", name="z")
            for i in range(nwarm):
                nc.tensor.matmul(wz, lhsT=warm_w, rhs=warm_x,
                                 start=(i == 0), stop=(i == nwarm - 1),
                                 perf_mode=DR)

            # ---------------- phase A ---------------------------------------
            sa = ctx.enter_context(ExitStack())
            wshp = sa.enter_context(tc.tile_pool(name="wshp", bufs=1))
            wsh13 = wshp.tile([128, 2, a13, hp], F32, tag="wsh13",
                              name="wsh13")
            qq2_p = sa.enter_context(tc.tile_pool(name="qq", bufs=1))
            with tc.tile_pool(name="scale", bufs=2) as sc_pool, \
                 tc.tile_pool(name="scale1", bufs=1) as sc1_pool:
                asum_a = sc1_pool.tile([128, 2, a13], F32, tag="asum_a",
                                       name="asum_a")
                for t, v in ((0, v1), (1, v3)):
                    for a in range(a13):
                        # raw-allgather columns land first
                        nc.sync.dma_start(wsh13[:, t, a, :rawsplit],
                                          v[:, a, :rawsplit])
                        nc.sync.dma_start(wsh13[:, t, a, rawsplit:],
                                          v[:, a, rawsplit:])
                        nc.vector.tensor_reduce(
                            asum_a[:, t, a:a + 1], wsh13[:, t, a], axis=AX.X,
                            op=ALU.add, apply_absolute_value=True)

                # raw bf16 chunk-0 allgather: first thing on the CC stream
                # after the framework barrier
                raw0 = sc1_pool.tile([128, 2, a13, c0w], BF16, tag="raw0",
                                     name="raw0")
                for t in (0, 1):
                    for a in range(a13):
                        nc.scalar.activation(raw0[:, t, a],
                                             wsh13[:, t, a, :c0w],
                                             AF.Copy, bias=0.0, scale=1.0)
                        nc.sync.dma_start(
                            agraw_in[0][a * 128:(a + 1) * 128, t, :],
                            raw0[:, t, a])
                nc.gpsimd.collective_compute(
                    "AllGather", ALU.bypass,
                    replica_groups=[list(range(n_cores))],
                    ins=[agraw_in[0].opt()],
                    outs=[agraw_out[0].opt()],
                )

                # w2-shard staged abs-sums
                nch2 = 8
                while sw % nch2:
                    nch2 //= 2
                c2w = sw // nch2
                asum2 = sc1_pool.tile([128, nch2], F32, tag="asum2",
                                      name="asum2")
                for c in range(nch2):
                    cs = slice(c * c2w, (c + 1) * c2w)
                    stg = sc_pool.tile([128, c2w], F32, tag="scstg",
                                       name="scstg")
                    nc.sync.dma_start(stg, wsh2[:, cs])
                    nc.vector.tensor_reduce(
                        asum2[:, c:c + 1], stg, axis=AX.X, op=ALU.add,
                        apply_absolute_value=True)
                part8 = sc1_pool.tile([128, 8], F32, tag="part8", name="part8")
                nc.vector.memset(part8, 0.0)
                nc.vector.tensor_reduce(part8[:, 0:1], asum_a[:, 0],
                                        axis=AX.X, op=ALU.add)
                nc.vector.tensor_reduce(part8[:, 1:2], asum_a[:, 1],
                                        axis=AX.X, op=ALU.add)
                nc.vector.tensor_reduce(part8[:, 2:3], asum2, axis=AX.X,
                                        op=ALU.add)
                par8 = sc1_pool.tile([128, 8], F32, tag="par8", name="par8")
                nc.gpsimd.partition_all_reduce(
                    par8, part8, channels=128, reduce_op=bass_isa.ReduceOp.add)
                nc.sync.dma_start(cc_in, par8[0:1, :])
                nc.gpsimd.collective_compute(
                    "AllReduce",
                    ALU.add,
                    replica_groups=[list(range(n_cores))],
                    ins=[cc_in.opt()],
                    outs=[cc_out.opt()],
                )

                # thresholds (emitted before the x-prep so the DVE computes
                # them the moment the AllReduce lands)
                g8 = sc1_pool.tile([1, 8], F32, tag="g8", name="g8")
                nc.sync.dma_start(g8, cc_out)
                gb = pc.tile([128, 8], F32, tag="gb", name="gb")
                nc.gpsimd.partition_broadcast(gb, g8)
                for t, name in enumerate(["w1", "w3", "w2"]):
                    for sgn in ("p", "n"):
                        bias[name + sgn] = pc.tile(
                            [128, 1], F32, tag=f"b_{name}{sgn}",
                            name=f"b_{name}{sgn}")
                        k = 0.5 / n_true if sgn == "p" else -0.5 / n_true
                        nc.vector.tensor_scalar(
                            bias[name + sgn], gb[:, t:t + 1], k, None,
                            ALU.mult,
                        )
                s23 = pc.tile([128, 1], F32, tag="s23", name="s23")
                nc.vector.tensor_mul(s23, bias["w3p"], bias["w2p"])

                def raw_prep(q, dst_tiles, stg_pool, q_pool):
                    """Ternarize a received raw bf16 chunk on the DVE."""
                    cw = sizes[q] * 128
                    rawv = agraw_out[q].rearrange(
                        "(k p) two h -> p k two h", p=128)
                    for t in range(2):
                        stg0 = stg_pool.tile([128, kd, cw], BF16,
                                             tag="rstg", name="rstg")
                        for k4 in range(0, kd, 4):
                            nc.sync.dma_start(stg0[:, k4:k4 + 4, :],
                                              rawv[:, k4:k4 + 4, t, :])
                        qa0 = q_pool.tile([128, kd, cw], w13_dt,
                                          tag="qra", name="qra")
                        qb0 = q_pool.tile([128, kd, cw], w13_dt,
                                          tag="qrb", name="qrb")
                        bp_ = bias["w1p" if t == 0 else "w3p"]
                        bn_ = bias["w1n" if t == 0 else "w3n"]
                        nc.vector.tensor_scalar(qa0, stg0, bp_, 2.0,
                                                ALU.is_ge, ALU.mult)
                        nc.vector.tensor_scalar(qb0, stg0, bn_, 2.0,
                                                ALU.is_lt, ALU.mult)
                        nc.vector.tensor_sub(dst_tiles[t][:, :, :cw],
                                             qa0, qb0)

                # chunk-0 local ternarize (before the x-prep: the DVE does
                # it the moment the thresholds and raw chunk are in)
                wqb0 = [c0p.tile([128, kd, c0w], w13_dt, tag=f"wq0{t}",
                                 name=f"wq0{t}") for t in range(2)]
                with tc.tile_pool(name="rst0", bufs=1) as r0_p, \
                     tc.tile_pool(name="qq0", bufs=1) as q0_p:
                    raw_prep(0, wqb0, r0_p, q0_p)

                def quantize(out_ap, stg_ap, bn, bp, dt, eng, fdim):
                    pr = stg_ap.shape[0]
                    if eng == "act":
                        qa = qq2_p.tile([128, fdim], dt, tag=f"qa{fdim}",
                                        name=f"qa{fdim}")
                        qb = qq2_p.tile([128, fdim], dt, tag=f"qb{fdim}",
                                        name=f"qb{fdim}")
                        fw = stg_ap.shape[-1]
                        nc.scalar.activation(qa[:pr, :fw], stg_ap, AF.Sign,
                                             bias=bn[:pr])
                        nc.scalar.activation(qb[:pr, :fw], stg_ap, AF.Sign,
                                             bias=bp[:pr])
                        nc.vector.tensor_add(out_ap, qa[:pr, :fw],
                                             qb[:pr, :fw])
                    else:
                        qa = qq2_p.tile([128, fdim], dt, tag=f"da{fdim}",
                                        name=f"da{fdim}")
                        qb = qq2_p.tile([128, fdim], dt, tag=f"db{fdim}",
                                        name=f"db{fdim}")
                        fw = stg_ap.shape[-1]
                        nc.vector.tensor_scalar(qa[:pr, :fw], stg_ap, bp[:pr],
                                                2.0, ALU.is_ge, ALU.mult)
                        nc.vector.tensor_scalar(qb[:pr, :fw], stg_ap, bn[:pr],
                                                2.0, ALU.is_lt, ALU.mult)
                        nc.vector.tensor_sub(out_ap, qa[:pr, :fw],
                                             qb[:pr, :fw])

                max_chw = max(cnt for _, cnt in chunks) * 128

                def tern_chunk(q, eng_for):
                    ct0, cnt = chunks[q]
                    chw = cnt * 128
                    hsl = slice(ct0 * 128, ct0 * 128 + chw)
                    for a in range(a13):
                        rs = slice(a * 128, (a + 1) * 128)
                        for t in (0, 1):
                            qt = qq2_p.tile([128, max_chw], w13_dt,
                                            tag="qt", name="qt")
                            quantize(qt[:, :chw], wsh13[:, t, a, hsl],
                                     bias["w1n" if t == 0 else "w3n"],
                                     bias["w1p" if t == 0 else "w3p"],
                                     w13_dt, eng_for(t), max_chw)
                            nc.sync.dma_start(agin13[q][rs, t, :],
                                              qt[:, :chw])
                    nc.gpsimd.collective_compute(
                        "AllGather", ALU.bypass,
                        replica_groups=[list(range(n_cores))],
                        ins=[agin13[q].opt()],
                        outs=[agout13[q].opt()],
                    )

                # chunk 1 entirely on the DVE, BEFORE the x-prep occupies it
                tern_chunk(1, lambda t: "dve")

                # ---- x load + exact fp8 two-term split, all on the DVE
                # (4x the ACT's element rate; the ACT runs the remaining
                # ternarize Signs meanwhile)
                with tc.tile_pool(name="xstg", bufs=2) as xstg_p:
                    for k in range(kd):
                        xstg = xstg_p.tile([128, m], F32, tag="xstg",
                                           name="xstg")
                        nc.sync.dma_start(xstg, xview[:, k, :])
                        nc.vector.tensor_copy(out=x_sb[:, k, :], in_=xstg)
                        nc.vector.tensor_sub(x_sb[:, kd + k, :],
                                             xstg, x_sb[:, k, :])

                # remaining chunks: Signs on the ACT, combine-adds on DVE
                for q in range(2, len(chunks)):
                    tern_chunk(q, lambda t: "act")

                # w2: staged re-read + quantize, alternate engines per chunk
                with tc.tile_pool(name="qstg", bufs=2) as qstg_p:
                    n2 = 8
                    while sw % n2:
                        n2 //= 2
                    c2 = sw // n2
                    for c in range(n2):
                        cs = slice(c * c2, (c + 1) * c2)
                        stg = qstg_p.tile([128, c2], F32, tag="q2stg",
                                          name="q2stg")
                        nc.sync.dma_start(stg, wsh2[:, cs])
                        qt = qq2_p.tile([128, c2], w2q_dt, tag="q2t",
                                        name="q2t")
                        quantize(qt, stg, bias["w2n"], bias["w2p"], w2q_dt,
                                 "act" if c % 2 == 0 else "dve", c2)
                        nc.sync.dma_start(agin2[:, cs], qt)
                nc.gpsimd.collective_compute(
                    "AllGather", ALU.bypass,
                    replica_groups=[list(range(n_cores))],
                    ins=[agin2.opt()],
                    outs=[agout2.opt()],
                )

            sa.close()   # free the resident w1/w3 shard

            # phase-B pools land in the space the shard freed
            sbp = ctx.enter_context(ExitStack())
            wq_p = sbp.enter_context(tc.tile_pool(name="wq", bufs=2))
            sl_p = sbp.enter_context(tc.tile_pool(name="slp", bufs=2))
            g_p = sbp.enter_context(tc.tile_pool(name="gp", bufs=2))

            # ---------------- phase B ---------------------------------------
            for q, (ct0, cnt) in enumerate(chunks):
                if q >= n_raw:
                    agov = agout13[q].rearrange(
                        "(k p) two h -> p k two h", p=128)
                for b0 in range(0, cnt, hb):
                    nh = min(hb, cnt - b0)
                    hw = nh * 128
                    habs = (ct0 + b0) * 128       # absolute h start
                    if q == 0:
                        wqb = wqb0
                    else:
                        wqb = []
                        for t in range(2):
                            wq_t = wq_p.tile([128, kd, hb * 128], w13_dt,
                                             tag=f"wq{t}", name=f"wq{t}")
                            # split across k: the strided 512B-line gather
                            # runs on parallel DMA queues
                            for k4 in range(0, kd, 4):
                                nc.sync.dma_start(
                                    wq_t[:, k4:k4 + 4, :hw],
                                    agov[:, k4:k4 + 4, t,
                                         b0 * 128:b0 * 128 + hw])
                            wqb.append(wq_t)

                    for hti in range(nh):
                        hs = slice(hti * 128, (hti + 1) * 128)
                        g_t = g_p.tile([128, m], BF16, tag="g_t", name="g_t")
                        zz = [[None] * mc, [None] * mc]
                        for t in range(2):
                            for mci in range(mc):
                                zz[t][mci] = zps.tile([128, 512], F32,
                                                      tag="z", name="z")
                            # j outer / mci inner: 8 consecutive matmuls
                            # share the same stationary weight pair
                            for j in range(kd // 2):
                                ks = slice(2 * j, 2 * j + 2)
                                kls = slice(kd + 2 * j, kd + 2 * j + 2)
                                for mci in range(mc):
                                    ms = slice(mci * 512, (mci + 1) * 512)
                                    nc.tensor.matmul(
                                        zz[t][mci], lhsT=wqb[t][:, ks, hs],
                                        rhs=x_sb[:, ks, ms],
                                        start=(j == 0), stop=False,
                                        perf_mode=DR,
                                    )
                                    nc.tensor.matmul(
                                        zz[t][mci], lhsT=wqb[t][:, ks, hs],
                                        rhs=x_sb[:, kls, ms],
                                        start=False,
                                        stop=(j == kd // 2 - 1),
                                        perf_mode=DR,
                                    )
                        for mci in range(mc):
                            ms = slice(mci * 512, (mci + 1) * 512)
                            sl = sl_p.tile([128, 512], BF16, tag="sl",
                                           name="sl")
                            nc.scalar.activation(sl, zz[0][mci], AF.Silu,
                                                 bias=0.0, scale=bias["w1p"])
                            # g = (z3 * s23) * silu(...)  in one DVE op
                            nc.vector.scalar_tensor_tensor(
                                g_t[:, ms], zz[1][mci], s23, sl,
                                ALU.mult, ALU.mult)
                        nc.sync.dma_start(
                            g_dram[habs + hti * 128:
                                   habs + (hti + 1) * 128, :], g_t)

            sbp.close()   # free wq/sl/g pools before phase C
            sx.close()    # free the phase-B PSUM pool

            # ---------------- phase C ---------------------------------------
            ndc = (d + 511) // 512
            with ExitStack() as scx:
                w2c_p = scx.enter_context(tc.tile_pool(name="w2c", bufs=ht))
                gq_p = scx.enter_context(tc.tile_pool(name="gq", bufs=3))
                y_p = scx.enter_context(tc.tile_pool(name="yp", bufs=2 * ndc))
                yps = scx.enter_context(
                    tc.tile_pool(name="yps", bufs=8, space="PSUM"))

                w2qk = []
                for k2 in range(ht):
                    t_ = w2c_p.tile([128, d], w2q_dt, tag="w2qk", name="w2qk")
                    nc.sync.dma_start(t_, agout2_rd[:, k2, :])
                    w2qk.append(t_)

                for mt in range(m // 128):
                    gq = gq_p.tile([128, ht, 128], BF16, tag="gq", name="gq")
                    # split the strided 256B-line gather across DMA queues
                    for kg in range(0, ht, 11):
                        ke = min(ht, kg + 11)
                        nc.sync.dma_start(
                            gq[:, kg:ke, :],
                            g_rd[:, kg:ke, mt * 128:(mt + 1) * 128])
                    yp4 = [yps.tile([128, 512], F32, tag="yps", name="yps")
                           for _ in range(ndc)]
                    for k2 in range(ht):
                        for di, dc0 in enumerate(range(0, d, 512)):
                            nd = min(512, d - dc0)
                            nc.tensor.matmul(
                                yp4[di][:, :nd],
                                lhsT=gq[:, k2, :],
                                rhs=w2qk[k2][:, dc0:dc0 + nd],
                                start=(k2 == 0), stop=(k2 == ht - 1),
                            )
                    for di, dc0 in enumerate(range(0, d, 512)):
                        nd = min(512, d - dc0)
                        ysb = y_p.tile([128, 512], BF16, tag="ysb", name="ysb")
                        nc.scalar.copy(ysb[:, :nd], yp4[di][:, :nd])
                        nc.sync.dma_start(
                            y[mt * 128:(mt + 1) * 128, dc0:dc0 + nd],
                            ysb[:, :nd])

    nc.compile()
    return nc


_NC_CACHE = {}


def _get_module():
    if "nc" not in _NC_CACHE:
        _NC_CACHE["nc"] = build_module()
    return _NC_CACHE["nc"]


def prep_inputs(x, w1, w3, w2, d=D, m=M, hp=HP, n_cores=N_CORES):
    """Host-side layout work: pad, transpose, shard, slice. No arithmetic."""
    h_real = w1.shape[0]
    x = np.ascontiguousarray(np.asarray(x, dtype=np.float32))
    xf = x.reshape(-1, d)
    w1t = np.zeros((d, hp), np.float32)
    w1t[:, :h_real] = np.asarray(w1, np.float32).T
    w3t = np.zeros((d, hp), np.float32)
    w3t[:, :h_real] = np.asarray(w3, np.float32).T
    w2t = np.zeros((hp, d), np.float32)
    w2t[:h_real, :] = np.asarray(w2, np.float32).T

    r13 = d // n_cores
    r2 = hp // n_cores
    sw = d * hp // (n_cores * 128)

    in_maps = []
    for c in range(n_cores):
        xc = np.ascontiguousarray(xf[c * m:(c + 1) * m].T)   # [d, m]
        in_maps.append({
            "xT": xc,
            "wsh1": np.ascontiguousarray(w1t[c * r13:(c + 1) * r13]),
            "wsh3": np.ascontiguousarray(w3t[c * r13:(c + 1) * r13]),
            "wsh2": np.ascontiguousarray(
                w2t[c * r2:(c + 1) * r2].reshape(128, sw)),
        })
    return in_maps


def kernel(x, w1, w3, w2):
    from concourse.bass_utils import run_bass_kernel_spmd

    nc = _get_module()
    in_maps = prep_inputs(x, w1, w3, w2)
    res = run_bass_kernel_spmd(nc, in_maps, core_ids=list(range(N_CORES)))
    _NC_CACHE["last_results"] = res
    yf = np.concatenate([np.asarray(r["y"]) for r in res.results], axis=0)
    return np.ascontiguousarray(
        yf.reshape(B, S, D).astype(np.float32))


# revision 47
# speedup vs baseline: 1.0625x; 1.0625x over previous
"""Trainium2 Bass kernel for nn_BinaryMLP (BitNet-ternary SwiGLU MLP).

reference math (fp32):
    s_i = mean(|w_i|)            (per-tensor scalar, i in {1,3,2})
    wq_i = clip(round(w_i/s_i), -1, 1) * s_i     (ternary * scale)
    h1 = x @ w1q.T ; h3 = x @ w3q.T
    y  = (silu(h1) * h3) @ w2q.T

Strategy (8 cores, data-parallel over the 16384 tokens):
  - host: pad H 5461->5504, transpose x / w1 / w3 / w2 into contraction-major
    layouts (pure layout work, no arithmetic), split tokens 8 ways, and give
    each core a distinct 1/8 row-slice of each weight tensor.
  - device (per core, identical SPMD program):
      phase A: w1/w3 shard resident in SBUF (read once); |w| partial sums
               (DVE abs-reduce) -> gpsimd partition-reduce -> tiny 8-core
               AllReduce -> ternarization thresholds +-s/2.  The first two
               h-chunks are allgathered RAW in bf16 (queued on the CC
               stream around the AllReduce, no threshold dependency) and
               ternarized locally on every core straight into SBUF weight
               tiles, so phase B starts as soon as the scales exist.
               Remaining chunks are ternarized at the source (fp8 {-2,0,2})
               and allgathered while phase B runs.  Meanwhile x is loaded
               and split exactly into two fp8 terms x = x_hi + x_lo
               (x_hi = fp8(x), x_lo = fp8(x - x_hi)) so phase B runs
               fp8xfp8 DoubleRow matmuls: same PE cycles as bf16, about
               half the PE power (the chip is power-brake limited).
      phase B: h1/h3 DoubleRow fp8 matmuls vs resident x_hi/x_lo (weight
               pairs stationary, 8 psum banks), fused epilogue
               g = silu((s1/2) z1) * (s3 s2/4 * z3) in one ACT + one DVE
               op -> bf16 -> DRAM.
      phase C: y[m,d] = sum_h g[h,m] t2[h,d], g stationary bf16, w2 fp8
               moving, fp32 PSUM, bf16 output (host upcasts losslessly
               concatenation-side).
  - host: concatenate the 8 token shards, reshape to [4, 4096, 2048].

All arithmetic (scales, ternarization, matmuls) happens on device; the host
only reshapes / transposes / pads / slices / concatenates / upcasts.
"""

import sys
from contextlib import ExitStack

import numpy as np

if "/opt/trn_rl_repo" not in sys.path:
    sys.path.insert(0, "/opt/trn_rl_repo")

import concourse.bass as bass  # noqa: E402,F401
import concourse.mybir as mybir  # noqa: E402
import concourse.tile as tile  # noqa: E402
from concourse import bacc  # noqa: E402
from concourse import bass_isa  # noqa: E402

F32 = mybir.dt.float32
BF16 = mybir.dt.bfloat16
FP8 = mybir.dt.float8e4
AF = mybir.ActivationFunctionType
ALU = mybir.AluOpType
AX = mybir.AxisListType
DR = mybir.MatmulPerfMode.DoubleRow

# Full problem geometry (hardcoded per contest rules).
B, S, D = 4, 4096, 2048
H_REAL = 5461
HP = 5504            # H padded to 43*128
N_CORES = 8
M = (B * S) // N_CORES   # tokens per core = 2048


def build_module(d=D, m=M, hp=HP, n_cores=N_CORES, h_real=H_REAL,
                 hb=4, w13_dt=FP8, w2q_dt=FP8, nwarm=16):
    """Build + compile the per-core SPMD Bass module."""
    kd = d // 128        # k-tiles over D
    ht = hp // 128       # h-tiles
    mc = m // 512        # m-chunks of 512 in phase B
    assert d % 128 == 0 and hp % 128 == 0 and m % 512 == 0
    n_true = h_real * d
    sw = d * hp // (n_cores * 128)   # w2-shard free elems per partition
    r13 = d // n_cores               # weight-slice rows (w1t/w3t)
    assert r13 % 128 == 0
    a13 = r13 // 128

    # h-tile chunks: chunks 0/1 go through the raw bf16 allgather (low
    # latency, threshold-free); 2+ are ternarized at source (fp8).
    sizes = [2, 3, 8, 8, 8, 8, 6]
    assert sum(sizes) == ht
    chunks = []
    t0 = 0
    for n in sizes:
        chunks.append((t0, n))
        t0 += n
    n_raw = 1
    c0w = sizes[0] * 128
    rawsplit = c0w           # wsh columns needed by the raw allgather

    nc = bacc.Bacc(
        "TRN2",
        target_bir_lowering=False,
        debug=False,
        num_devices=n_cores,
    )
    xT = nc.dram_tensor("xT", [d, m], F32, kind="ExternalInput").ap()
    wsh1 = nc.dram_tensor("wsh1", [r13, hp], F32, kind="ExternalInput").ap()
    wsh3 = nc.dram_tensor("wsh3", [r13, hp], F32, kind="ExternalInput").ap()
    wsh2 = nc.dram_tensor("wsh2", [128, sw], F32, kind="ExternalInput").ap()
    y = nc.dram_tensor("y", [m, d], BF16, kind="ExternalOutput").ap()

    xview = xT.rearrange("(k p) m -> p k m", p=128)
    v1 = wsh1.rearrange("(a p) h -> p a h", p=128)   # [128, a13, hp]
    v3 = wsh3.rearrange("(a p) h -> p a h", p=128)

    with tile.TileContext(nc) as tc:
        with ExitStack() as ctx:
            dram = ctx.enter_context(tc.tile_pool(name="dram", bufs=1, space="DRAM"))
            g_dram = dram.tile([hp, m], BF16, tag="g", name="g")
            cc_in = dram.tile([1, 8], F32, tag="cc_in", name="cc_in")
            cc_out = dram.tile([1, 8], F32, tag="cc_out", name="cc_out")
            g_rd = g_dram.rearrange("(k p) m -> p k m", p=128)

            agraw_in = []
            agraw_out = []
            for q in range(n_raw):
                cw = sizes[q] * 128
                agraw_in.append(dram.tile([r13, 2, cw], BF16,
                                          tag=f"agri{q}", name=f"agri{q}"))
                agraw_out.append(dram.tile([d, 2, cw], BF16,
                                           tag=f"agro{q}", name=f"agro{q}"))
            agin13 = [None] * n_raw
            agout13 = [None] * n_raw
            for q, (ct0, cnt) in enumerate(chunks):
                if q < n_raw:
                    continue
                agin13.append(dram.tile(
                    [r13, 2, cnt * 128], w13_dt,
                    tag=f"agi{q}", name=f"agi{q}"))
                agout13.append(dram.tile(
                    [d, 2, cnt * 128], w13_dt,
                    tag=f"ago{q}", name=f"ago{q}"))
            agin2 = dram.tile([128, sw], w2q_dt, tag="agi2", name="agi2")
            agout2 = dram.tile([hp, d], w2q_dt, tag="ago2", name="ago2")
            agout2_rd = agout2.rearrange("(k p) dd -> p k dd", p=128)

            pc = ctx.enter_context(tc.tile_pool(name="pconst", bufs=1))
            bias = {}
            # x_hi / x_lo fp8, resident through phase B (dead but allocated
            # in phase C — pool stack is LIFO)
            xp = ctx.enter_context(tc.tile_pool(name="xp", bufs=1))
            x_sb = xp.tile([128, 2 * kd, m], FP8, tag="x_sb", name="x_sb")
            # chunk-0 ternary tiles (locally quantized during phase A)
            c0p = ctx.enter_context(tc.tile_pool(name="c0p", bufs=1))
            sx = ctx.enter_context(ExitStack())
            zps = sx.enter_context(tc.tile_pool(name="zps", bufs=8,
                                                space="PSUM"))

            # ---------------- warmup: keep the PE spinning from t=0 ---------
            warm_w = pc.tile([128, 2, 128], FP8, tag="warm_w", name="warm_w")
            warm_x = pc.tile([128, 2, 512], FP8, tag="warm_x", name="warm_x")
            nc.vector.memset(warm_w, 0.125)
            nc.vector.memset(warm_x, 0.125)
            wz = zps.tile([128, 512], F32, tag="z", name="z")
            for i in range(nwarm):
                nc.tensor.matmul(wz, lhsT=warm_w, rhs=warm_x,
                                 start=(i == 0), stop=(i == nwarm - 1),
                                 perf_mode=DR)

            # ---------------- phase A ---------------------------------------
            sa = ctx.enter_context(ExitStack())
            wshp = sa.enter_context(tc.tile_pool(name="wshp", bufs=1))
            wsh13 = wshp.tile([128, 2, a13, hp], F32, tag="wsh13",
                              name="wsh13")
            qq2_p = sa.enter_context(tc.tile_pool(name="qq", bufs=1))
            with tc.tile_pool(name="scale", bufs=2) as sc_pool, \
                 tc.tile_pool(name="scale1", bufs=1) as sc1_pool:
                asum_a = sc1_pool.tile([128, 2, a13], F32, tag="asum_a",
                                       name="asum_a")
                for t, v in ((0, v1), (1, v3)):
                    for a in range(a13):
                        # raw-allgather columns land first
                        nc.sync.dma_start(wsh13[:, t, a, :rawsplit],
                                          v[:, a, :rawsplit])
                        nc.sync.dma_start(wsh13[:, t, a, rawsplit:],
                                          v[:, a, rawsplit:])
                        nc.vector.tensor_reduce(
                            asum_a[:, t, a:a + 1], wsh13[:, t, a], axis=AX.X,
                            op=ALU.add, apply_absolute_value=True)

                # raw bf16 chunk-0 allgather: first thing on the CC stream
                # after the framework barrier
                raw0 = sc1_pool.tile([128, 2, a13, c0w], BF16, tag="raw0",
                                     name="raw0")
                for t in (0, 1):
                    for a in range(a13):
                        nc.scalar.activation(raw0[:, t, a],
                                             wsh13[:, t, a, :c0w],
                                             AF.Copy, bias=0.0, scale=1.0)
                        nc.sync.dma_start(
                            agraw_in[0][a * 128:(a + 1) * 128, t, :],
                            raw0[:, t, a])
                nc.gpsimd.collective_compute(
                    "AllGather", ALU.bypass,
                    replica_groups=[list(range(n_cores))],
                    ins=[agraw_in[0].opt()],
                    outs=[agraw_out[0].opt()],
                )

                # w2-shard staged abs-sums
                nch2 = 8
                while sw % nch2:
                    nch2 //= 2
                c2w = sw // nch2
                asum2 = sc1_pool.tile([128, nch2], F32, tag="asum2",
                                      name="asum2")
                for c in range(nch2):
                    cs = slice(c * c2w, (c + 1) * c2w)
                    stg = sc_pool.tile([128, c2w], F32, tag="scstg",
                                       name="scstg")
                    nc.sync.dma_start(stg, wsh2[:, cs])
                    nc.vector.tensor_reduce(
                        asum2[:, c:c + 1], stg, axis=AX.X, op=ALU.add,
                        apply_absolute_value=True)
                part8 = sc1_pool.tile([128, 8], F32, tag="part8", name="part8")
                nc.vector.memset(part8, 0.0)
                nc.vector.tensor_reduce(part8[:, 0:1], asum_a[:, 0],
                                        axis=AX.X, op=ALU.add)
                nc.vector.tensor_reduce(part8[:, 1:2], asum_a[:, 1],
                                        axis=AX.X, op=ALU.add)
                nc.vector.tensor_reduce(part8[:, 2:3], asum2, axis=AX.X,
                                        op=ALU.add)
                par8 = sc1_pool.tile([128, 8], F32, tag="par8", name="par8")
                nc.gpsimd.partition_all_reduce(
                    par8, part8, channels=128, reduce_op=bass_isa.ReduceOp.add)
                nc.sync.dma_start(cc_in, par8[0:1, :])
                nc.gpsimd.collective_compute(
                    "AllReduce",
                    ALU.add,
                    replica_groups=[list(range(n_cores))],
                    ins=[cc_in.opt()],
                    outs=[cc_out.opt()],
                )

                # thresholds (emitted before the x-prep so the DVE computes
                # them the moment the AllReduce lands)
                g8 = sc1_pool.tile([1, 8], F32, tag="g8", name="g8")
                nc.sync.dma_start(g8, cc_out)
                gb = pc.tile([128, 8], F32, tag="gb", name="gb")
                nc.gpsimd.partition_broadcast(gb, g8)
                for t, name in enumerate(["w1", "w3", "w2"]):
                    for sgn in ("p", "n"):
                        bias[name + sgn] = pc.tile(
                            [128, 1], F32, tag=f"b_{name}{sgn}",
                            name=f"b_{name}{sgn}")
                        k = 0.5 / n_true if sgn == "p" else -0.5 / n_true
                        nc.vector.tensor_scalar(
                            bias[name + sgn], gb[:, t:t + 1], k, None,
                            ALU.mult,
                        )
                s23 = pc.tile([128, 1], F32, tag="s23", name="s23")
                nc.vector.tensor_mul(s23, bias["w3p"], bias["w2p"])

                def raw_prep(q, dst_tiles, stg_pool, q_pool):
                    """Ternarize a received raw bf16 chunk on the DVE."""
                    cw = sizes[q] * 128
                    rawv = agraw_out[q].rearrange(
                        "(k p) two h -> p k two h", p=128)
                    for t in range(2):
                        stg0 = stg_pool.tile([128, kd, cw], BF16,
                                             tag="rstg", name="rstg")
                        for k4 in range(0, kd, 4):
                            nc.sync.dma_start(stg0[:, k4:k4 + 4, :],
                                              rawv[:, k4:k4 + 4, t, :])
                        qa0 = q_pool.tile([128, kd, cw], w13_dt,
                                          tag="qra", name="qra")
                        qb0 = q_pool.tile([128, kd, cw], w13_dt,
                                          tag="qrb", name="qrb")
                        bp_ = bias["w1p" if t == 0 else "w3p"]
                        bn_ = bias["w1n" if t == 0 else "w3n"]
                        nc.vector.tensor_scalar(qa0, stg0, bp_, 2.0,
                                                ALU.is_ge, ALU.mult)
                        nc.vector.tensor_scalar(qb0, stg0, bn_, 2.0,
                                                ALU.is_lt, ALU.mult)
                        nc.vector.tensor_sub(dst_tiles[t][:, :, :cw],
                                             qa0, qb0)

                # chunk-0 local ternarize (before the x-prep: the DVE does
                # it the moment the thresholds and raw chunk are in)
                wqb0 = [c0p.tile([128, kd, c0w], w13_dt, tag=f"wq0{t}",
                                 name=f"wq0{t}") for t in range(2)]
                with tc.tile_pool(name="rst0", bufs=1) as r0_p, \
                     tc.tile_pool(name="qq0", bufs=1) as q0_p:
                    raw_prep(0, wqb0, r0_p, q0_p)

                def quantize(out_ap, stg_ap, bn, bp, dt, eng, fdim):
                    pr = stg_ap.shape[0]
                    if eng == "act":
                        qa = qq2_p.tile([128, fdim], dt, tag=f"qa{fdim}",
                                        name=f"qa{fdim}")
                        qb = qq2_p.tile([128, fdim], dt, tag=f"qb{fdim}",
                                        name=f"qb{fdim}")
                        fw = stg_ap.shape[-1]
                        nc.scalar.activation(qa[:pr, :fw], stg_ap, AF.Sign,
                                             bias=bn[:pr])
                        nc.scalar.activation(qb[:pr, :fw], stg_ap, AF.Sign,
                                             bias=bp[:pr])
                        nc.vector.tensor_add(out_ap, qa[:pr, :fw],
                                             qb[:pr, :fw])
                    else:
                        qa = qq2_p.tile([128, fdim], dt, tag=f"da{fdim}",
                                        name=f"da{fdim}")
                        qb = qq2_p.tile([128, fdim], dt, tag=f"db{fdim}",
                                        name=f"db{fdim}")
                        fw = stg_ap.shape[-1]
                        nc.vector.tensor_scalar(qa[:pr, :fw], stg_ap, bp[:pr],
                                                2.0, ALU.is_ge, ALU.mult)
                        nc.vector.tensor_scalar(qb[:pr, :fw], stg_ap, bn[:pr],
                                                2.0, ALU.is_lt, ALU.mult)
                        nc.vector.tensor_sub(out_ap, qa[:pr, :fw],
                                             qb[:pr, :fw])

                max_chw = max(cnt for _, cnt in chunks) * 128

                def tern_chunk(q, eng_for):
                    ct0, cnt = chunks[q]
                    chw = cnt * 128
                    hsl = slice(ct0 * 128, ct0 * 128 + chw)
                    for a in range(a13):
                        rs = slice(a * 128, (a + 1) * 128)
                        for t in (0, 1):
                            qt = qq2_p.tile([128, max_chw], w13_dt,
                                            tag="qt", name="qt")
                            quantize(qt[:, :chw], wsh13[:, t, a, hsl],
                                     bias["w1n" if t == 0 else "w3n"],
                                     bias["w1p" if t == 0 else "w3p"],
                                     w13_dt, eng_for(t), max_chw)
                            nc.sync.dma_start(agin13[q][rs, t, :],
                                              qt[:, :chw])
                    nc.gpsimd.collective_compute(
                        "AllGather", ALU.bypass,
                        replica_groups=[list(range(n_cores))],
                        ins=[agin13[q].opt()],
                        outs=[agout13[q].opt()],
                    )

                # chunk 1 entirely on the DVE, BEFORE the x-prep occupies it
                tern_chunk(1, lambda t: "dve")

                # ---- x load + exact fp8 two-term split (hi on ACT, lo on
                # DVE, so phase B's epilogue engines free up evenly)
                with tc.tile_pool(name="xstg", bufs=2) as xstg_p:
                    for k in range(kd):
                        xstg = xstg_p.tile([128, m], F32, tag="xstg",
                                           name="xstg")
                        nc.sync.dma_start(xstg, xview[:, k, :])
                        nc.scalar.activation(x_sb[:, k, :], xstg, AF.Copy,
                                             bias=0.0, scale=1.0)
                        nc.vector.tensor_sub(x_sb[:, kd + k, :],
                                             xstg, x_sb[:, k, :])

                # remaining chunks: w1 on ACT, w3 on DVE
                for q in range(2, len(chunks)):
                    tern_chunk(q, lambda t: "act" if t == 0 else "dve")

                # w2: staged re-read + quantize, alternate engines per chunk
                with tc.tile_pool(name="qstg", bufs=2) as qstg_p:
                    n2 = 8
                    while sw % n2:
                        n2 //= 2
                    c2 = sw // n2
                    for c in range(n2):
                        cs = slice(c * c2, (c + 1) * c2)
                        stg = qstg_p.tile([128, c2], F32, tag="q2stg",
                                          name="q2stg")
                        nc.sync.dma_start(stg, wsh2[:, cs])
                        qt = qq2_p.tile([128, c2], w2q_dt, tag="q2t",
                                        name="q2t")
                        quantize(qt, stg, bias["w2n"], bias["w2p"], w2q_dt,
                                 "act" if c % 2 == 0 else "dve", c2)
                        nc.sync.dma_start(agin2[:, cs], qt)
                nc.gpsimd.collective_compute(
                    "AllGather", ALU.bypass,
                    replica_groups=[list(range(n_cores))],
                    ins=[agin2.opt()],
                    outs=[agout2.opt()],
                )

            sa.close()   # free the resident w1/w3 shard

            # phase-B pools land in the space the shard freed
            sbp = ctx.enter_context(ExitStack())
            wq_p = sbp.enter_context(tc.tile_pool(name="wq", bufs=2))
            sl_p = sbp.enter_context(tc.tile_pool(name="slp", bufs=2))
            g_p = sbp.enter_context(tc.tile_pool(name="gp", bufs=2))

            # ---------------- phase B ---------------------------------------
            for q, (ct0, cnt) in enumerate(chunks):
                if q >= n_raw:
                    agov = agout13[q].rearrange(
                        "(k p) two h -> p k two h", p=128)
                for b0 in range(0, cnt, hb):
                    nh = min(hb, cnt - b0)
                    hw = nh * 128
                    habs = (ct0 + b0) * 128       # absolute h start
                    if q == 0:
                        wqb = wqb0
                    else:
                        wqb = []
                        for t in range(2):
                            wq_t = wq_p.tile([128, kd, hb * 128], w13_dt,
                                             tag=f"wq{t}", name=f"wq{t}")
                            # split across k: the strided 512B-line gather
                            # runs on parallel DMA queues
                            for k4 in range(0, kd, 4):
                                nc.sync.dma_start(
                                    wq_t[:, k4:k4 + 4, :hw],
                                    agov[:, k4:k4 + 4, t,
                                         b0 * 128:b0 * 128 + hw])
                            wqb.append(wq_t)

                    for hti in range(nh):
                        hs = slice(hti * 128, (hti + 1) * 128)
                        g_t = g_p.tile([128, m], BF16, tag="g_t", name="g_t")
                        zz = [[None] * mc, [None] * mc]
                        for t in range(2):
                            for mci in range(mc):
                                zz[t][mci] = zps.tile([128, 512], F32,
                                                      tag="z", name="z")
                            # j outer / mci inner: 8 consecutive matmuls
                            # share the same stationary weight pair
                            for j in range(kd // 2):
                                ks = slice(2 * j, 2 * j + 2)
                                kls = slice(kd + 2 * j, kd + 2 * j + 2)
                                for mci in range(mc):
                                    ms = slice(mci * 512, (mci + 1) * 512)
                                    nc.tensor.matmul(
                                        zz[t][mci], lhsT=wqb[t][:, ks, hs],
                                        rhs=x_sb[:, ks, ms],
                                        start=(j == 0), stop=False,
                                        perf_mode=DR,
                                    )
                                    nc.tensor.matmul(
                                        zz[t][mci], lhsT=wqb[t][:, ks, hs],
                                        rhs=x_sb[:, kls, ms],
                                        start=False,
                                        stop=(j == kd // 2 - 1),
                                        perf_mode=DR,
                                    )
                        for mci in range(mc):
                            ms = slice(mci * 512, (mci + 1) * 512)
                            sl = sl_p.tile([128, 512], BF16, tag="sl",
                                           name="sl")
                            nc.scalar.activation(sl, zz[0][mci], AF.Silu,
                                                 bias=0.0, scale=bias["w1p"])
                            # g = (z3 * s23) * silu(...)  in one DVE op
                            nc.vector.scalar_tensor_tensor(
                                g_t[:, ms], zz[1][mci], s23, sl,
                                ALU.mult, ALU.mult)
                        nc.sync.dma_start(
                            g_dram[habs + hti * 128:
                                   habs + (hti + 1) * 128, :], g_t)

            sbp.close()   # free wq/sl/g pools before phase C
            sx.close()    # free the phase-B PSUM pool

            # ---------------- phase C ---------------------------------------
            ndc = (d + 511) // 512
            with ExitStack() as scx:
                w2c_p = scx.enter_context(tc.tile_pool(name="w2c", bufs=ht))
                gq_p = scx.enter_context(tc.tile_pool(name="gq", bufs=3))
                y_p = scx.enter_context(tc.tile_pool(name="yp", bufs=2 * ndc))
                yps = scx.enter_context(
                    tc.tile_pool(name="yps", bufs=8, space="PSUM"))

                w2qk = []
                for k2 in range(ht):
                    t_ = w2c_p.tile([128, d], w2q_dt, tag="w2qk", name="w2qk")
                    nc.sync.dma_start(t_, agout2_rd[:, k2, :])
                    w2qk.append(t_)

                for mt in range(m // 128):
                    gq = gq_p.tile([128, ht, 128], BF16, tag="gq", name="gq")
                    # split the strided 256B-line gather across DMA queues
                    for kg in range(0, ht, 11):
                        ke = min(ht, kg + 11)
                        nc.sync.dma_start(
                            gq[:, kg:ke, :],
                            g_rd[:, kg:ke, mt * 128:(mt + 1) * 128])
                    yp4 = [yps.tile([128, 512], F32, tag="yps", name="yps")
                           for _ in range(ndc)]
                    for k2 in range(ht):
                        for di, dc0 in enumerate(range(0, d, 512)):
                            nd = min(512, d - dc0)
                            nc.tensor.matmul(
                                yp4[di][:, :nd],
                                lhsT=gq[:, k2, :],
                                rhs=w2qk[k2][:, dc0:dc0 + nd],
                                start=(k2 == 0), stop=(k2 == ht - 1),
                            )
                    for di, dc0 in enumerate(range(0, d, 512)):
                        nd = min(512, d - dc0)
                        ysb = y_p.tile([128, 512], BF16, tag="ysb", name="ysb")
                        nc.scalar.copy(ysb[:, :nd], yp4[di][:, :nd])
                        nc.sync.dma_start(
                            y[mt * 128:(mt + 1) * 128, dc0:dc0 + nd],
                            ysb[:, :nd])

    nc.compile()
    return nc


_NC_CACHE = {}


def _get_module():
    if "nc" not in _NC_CACHE:
        _NC_CACHE["nc"] = build_module()
    return _NC_CACHE["nc"]


def prep_inputs(x, w1, w3, w2, d=D, m=M, hp=HP, n_cores=N_CORES):
    """Host-side layout work: pad, transpose, shard, slice. No arithmetic."""
    h_real = w1.shape[0]
    x = np.ascontiguousarray(np.asarray(x, dtype=np.float32))
    xf = x.reshape(-1, d)
    w1t = np.zeros((d, hp), np.float32)
    w1t[:, :h_real] = np.asarray(w1, np.float32).T
    w3t = np.zeros((d, hp), np.float32)
    w3t[:, :h_real] = np.asarray(w3, np.float32).T
    w2t = np.zeros((hp, d), np.float32)
    w2t[:h_real, :] = np.asarray(w2, np.float32).T

    r13 = d // n_cores
    r2 = hp // n_cores
    sw = d * hp // (n_cores * 128)

    in_maps = []
    for c in range(n_cores):
        xc = np.ascontiguousarray(xf[c * m:(c + 1) * m].T)   # [d, m]
        in_maps.append({
            "xT": xc,
            "wsh1": np.ascontiguousarray(w1t[c * r13:(c + 1) * r13]),
            "wsh3": np.ascontiguousarray(w3t[c * r13:(c + 1) * r13]),
            "wsh2": np.ascontiguousarray(
                w2t[c * r2:(c + 1) * r2].reshape(128, sw)),
        })
    return in_maps


def kernel(x, w1, w3, w2):
    from concourse.bass_utils import run_bass_kernel_spmd

    nc = _get_module()
    in_maps = prep_inputs(x, w1, w3, w2)
    res = run_bass_kernel_spmd(nc, in_maps, core_ids=list(range(N_CORES)))
    _NC_CACHE["last_results"] = res
    yf = np.concatenate([np.asarray(r["y"]) for r in res.results], axis=0)
    return np.ascontiguousarray(
        yf.reshape(B, S, D).astype(np.float32))
